# revision 2
# baseline (speedup 1.0000x reference)
"""Fused OT-DTW l2 cost-matrix kernel for Trainium2 (8 NeuronCores, SPMD).

mat_cost[i,j] = sum_{t,p,d} pi[cl(i)][t,p] * (X[i,t,d] - Y[j,p,d])^2
             = C1[i] + C2[cl(i), j] - 2 * C3[i,j]

with C3[i,j] = sum_{p,d} XP[i,p,d] * Y[j,p,d],  XP[i] = X[i].T @ pi[cl(i)].
The device computes the heavy parts (XP: ~69 GFLOP, C3: ~137 GFLOP) in
fp8e4m3 (pi is 0/1 so fp8 is exact for it; X/Y quantization error washes
out over the 65536-term contraction). The tiny rank-1 corrections C1/C2
(<0.2% of FLOPs) are applied on the host in fp32.

Sharding (4 row-groups x 2 p-halves): core k = 2g + h takes X rows
[256g, 256g+256) and contraction half p in [256h, 256h+256). Each core
emits the partial C3 over its p-half; the host adds the two partials per
group. This halves the Y stream per core (33.5MB vs 67MB, now under the
stage-B PE time) and splits stage A across cores with zero duplication,
putting per-core PE work at the global fp8 roofline (~164us).

Both stages run fp8 DoubleRow (contraction 256/instr). Stage A computes
XPT_i[d,p_half] = X[i].T @ pi_cl(i) as 2 accumulating DR matmuls per row
(t-pairs of 128-chunks; class picked via register-offset AP into resident
pi), then corner-turns PSUM->SBUF into xpt[d, p, i] fp8 with packed 4-row
casts split across DVE/ACT. Stage B accumulates C3[i, j] over 128 DR
p-pairs x 2 i-chunks x 2 j-halves: lhsT = xpt[:, p:p+2, ic], rhs = yt
tile [d, 8p, 1024j] streamed through an 8-buffer ring. A short scratch-
matmul burst first warms the PE clock-gate.
"""

import os
import sys
import types

import numpy as np
import ml_dtypes

NX, NY, T, TP, D, C = 1024, 1024, 512, 512, 128, 8
N_CORES = 8
GX = 4                      # row groups
R = NX // GX                # 256 rows per core
PL = TP // 2                # 256 local p (contraction half)
TC = T // 128               # 4 t-chunks
PG = 8                      # p-tiles per Y DMA
BF16 = ml_dtypes.bfloat16
F8 = ml_dtypes.float8_e4m3fn


def _ensure_axon_hooks():
    """concourse.bass_utils imports antenv.axon_hooks when tracing under
    axon; some images lack that submodule. Provide it, and register the
    NTFF profile hook if the boot path didn't."""
    try:
        import antenv
    except ImportError:
        return
    try:
        from antenv import axon_hooks  # noqa: F401
    except ImportError:
        mod = types.ModuleType("antenv.axon_hooks")
        mod._hook = None

        def _set(h):
            mod._hook = h

        def _get():
            return mod._hook

        mod.set_axon_ntff_profile_hook = _set
        mod.get_axon_ntff_profile_hook = _get
        sys.modules["antenv.axon_hooks"] = mod
        antenv.axon_hooks = mod
    from antenv.axon_hooks import (
        get_axon_ntff_profile_hook,
        set_axon_ntff_profile_hook,
    )

    if get_axon_ntff_profile_hook() is None:
        try:
            from trn_agent_boot.trn_boot import _ntff_profile_via_ctypes

            hook = _ntff_profile_via_ctypes("/opt/axon/libaxon_pjrt.so")
            if hook is not None:
                set_axon_ntff_profile_hook(hook)
        except Exception:
            pass


_ensure_axon_hooks()

import concourse.bass as bass  # noqa: E402
import concourse.tile as tile  # noqa: E402
from concourse import bacc, mybir  # noqa: E402
from concourse.bass_utils import run_bass_kernel_spmd  # noqa: E402

_PROGRAM_CACHE = {}
LAST_RUN = None  # BassKernelResults of the most recent kernel() call


def _build_program():
    if "nc" in _PROGRAM_CACHE:
        return _PROGRAM_CACHE["nc"]
    f8 = mybir.dt.float8e4
    f32 = mybir.dt.float32
    i32 = mybir.dt.int32
    DR = mybir.MatmulPerfMode.DoubleRow
    nc = bacc.Bacc("TRN2", target_bir_lowering=False, debug=False,
                   num_devices=N_CORES)
    xs = nc.dram_tensor("xs", [R, 128, TC, D], f8, kind="ExternalInput").ap()
    pi_d = nc.dram_tensor("pi_d", [128, TC, C * PL], f8, kind="ExternalInput").ap()
    offs = nc.dram_tensor("offs", [1, R], i32, kind="ExternalInput").ap()
    yt = nc.dram_tensor("yt", [D, PL, NY], f8, kind="ExternalInput").ap()
    c3 = nc.dram_tensor("c3", [R, NY], f32, kind="ExternalOutput").ap()

    with tile.TileContext(nc) as tc:
        with (
            tc.tile_pool(name="xpt", bufs=1) as xpt_pool,
            tc.tile_pool(name="xin", bufs=4) as xin_pool,
            tc.tile_pool(name="pisb", bufs=1) as pi_pool,
            tc.tile_pool(name="yin", bufs=8) as y_pool,
            tc.tile_pool(name="outsb", bufs=1) as out_pool,
        ):
            # Resident transposed XP for all local rows: [d, p, i] fp8
            # (p-major pairs for DoubleRow lhsT interleave).
            xpt = xpt_pool.tile([D, PL, R], f8)

            # PE warmup: ~14 matmuls on scratch data at t=0, overlapping the
            # first input DMAs, so the HAM clock-gate reaches 8/8 before the
            # real matmuls start (values never read; NaNs harmless).
            with (
                tc.tile_pool(name="warm", bufs=1) as warm_pool,
                tc.tile_pool(name="warmps", bufs=1, space="PSUM") as warmps_pool,
            ):
                wsrc = warm_pool.tile([128, 512], f8)
                wacc = warmps_pool.tile([128, 512], f32)
                nc.gpsimd.memset(wsrc[:], 0.0)
                for w in range(14):
                    nc.tensor.matmul(wacc[:], wsrc[:, 0:128], wsrc[:],
                                     start=True, stop=True)

            # ---- Stage A: XPT_i = X[i].T @ pi[cl(i)][:, p-half] ----
            # pi resident in SBUF (all classes' half, 1MB); per-row class
            # selected via register offset read from `offs` (= classe * PL).
            off_sb = pi_pool.tile([1, R], i32)
            nc.sync.dma_start(off_sb[:], offs[:])
            pi_sb = pi_pool.tile([128, TC, C * PL], f8)
            for c in range(TC):   # per-chunk loads: first matmul only waits c=0
                nc.sync.dma_start(pi_sb[:, c, :], pi_d[:, c, :])
            XB, CB = 4, 4   # xs rows per DMA, rows per packed cast
            with tc.tile_pool(name="psA", bufs=2, space="PSUM") as psA_pool:
                for i in range(R):
                    if i % XB == 0:
                        xt = xin_pool.tile([128, XB, TC, D], f8, tag="xt")
                        nc.sync.dma_start(
                            xt[:], xs[i:i + XB].rearrange("a t c d -> t a c d"))
                    if i % CB == 0:
                        acc = psA_pool.tile([D, CB, PL], f32)  # 2 PSUM banks
                    xv = xt[:, i % XB]
                    off = nc.values_load(
                        off_sb[0:1, i:i + 1], engines=[mybir.EngineType.PE],
                        min_val=0, max_val=(C - 1) * PL,
                        skip_runtime_bounds_check=True)
                    for h in range(TC // 2):
                        nc.tensor.matmul(
                            acc[:, i % CB, :],
                            xv[:, 2 * h:2 * h + 2, :],
                            pi_sb[:, 2 * h:2 * h + 2, bass.ds(off, PL)],
                            start=(h == 0), stop=(h == TC // 2 - 1),
                            perf_mode=DR,
                        )
                    if i % CB == CB - 1:
                        # Packed corner-turn: psum[d, 4, p] -> xpt[d, p, i..i+3]
                        # (4B inner runs @128B stride; 4x the 1B-run rate).
                        # Split p-range across DVE and ACT to halve latency.
                        g0 = i - (CB - 1)
                        h2 = PL // 2
                        s0 = acc[:, :, 0:h2].rearrange("d k p -> d p k")
                        s1 = acc[:, :, h2:PL].rearrange("d k p -> d p k")
                        nc.vector.tensor_copy(xpt[:, 0:h2, g0:g0 + CB], s0)
                        nc.scalar.copy(xpt[:, h2:PL, g0:g0 + CB], s1)

            # ---- Stage B: C3[i, j] partial = sum_{p half} XPT YT, DR pairs ----
            with tc.tile_pool(name="psB", bufs=1, space="PSUM") as psB_pool:
                accs = [[psB_pool.tile([128, 512], f32) for _ in range(2)]
                        for _ in range(2)]   # [i-chunk][j-half]
                for g in range(PL // PG):
                    ytile = y_pool.tile([D, PG, NY], f8)
                    nc.sync.dma_start(ytile[:], yt[:, g * PG:(g + 1) * PG, :])
                    for s in range(PG // 2):
                        p = g * PG + 2 * s
                        st, sp = (p == 0), (p == PL - 2)
                        rhs = ytile[:, 2 * s:2 * s + 2, :]
                        for ic in range(2):
                            lhsT = xpt[:, p:p + 2, 128 * ic:128 * ic + 128]
                            for jh in range(2):
                                nc.tensor.matmul(
                                    accs[ic][jh],
                                    lhsT, rhs[:, :, 512 * jh:512 * jh + 512],
                                    start=st, stop=sp, perf_mode=DR)

            out_sb = out_pool.tile([128, 2, NY], f32)
            nc.vector.tensor_copy(out_sb[:, 0, 0:512], accs[0][0][:])
            nc.scalar.copy(out_sb[:, 0, 512:1024], accs[0][1][:])
            nc.vector.tensor_copy(out_sb[:, 1, 0:512], accs[1][0][:])
            nc.scalar.copy(out_sb[:, 1, 512:1024], accs[1][1][:])
            nc.sync.dma_start(c3.rearrange("(ic q) j -> q ic j", q=128), out_sb[:])

    nc.compile()
    _PROGRAM_CACHE["nc"] = nc
    return nc


def kernel(X, Y, pi, classe):
    global LAST_RUN
    assert X.shape == (NX, T, D) and Y.shape == (NY, TP, D)
    assert pi.shape == (C, T, TP) and classe.shape == (NX,)
    X = np.asarray(X, dtype=np.float32)
    Y = np.asarray(Y, dtype=np.float32)
    pi = np.asarray(pi, dtype=np.float32)
    classe = np.asarray(classe)

    nc = _build_program()

    # Host-side sharding + layout prep (all-contiguous device DMAs).
    yt_full = np.ascontiguousarray(Y.transpose(2, 1, 0).astype(F8))  # [d,p,j]
    pi8 = pi.astype(F8)
    in_maps = []
    for g in range(GX):
        rows = slice(g * R, (g + 1) * R)
        xk = X[rows].astype(F8)                        # [R, T, D]
        xk = np.ascontiguousarray(
            xk.reshape(R, TC, 128, D).transpose(0, 2, 1, 3))
        offs = (classe[rows].astype(np.int32) * PL)[None, :]
        for h in range(2):
            # pi_d[t%128, c, cls*PL + p] = pi[cls, t, 256h + p]
            pi_d = np.ascontiguousarray(
                pi8[:, :, h * PL:(h + 1) * PL]
                .reshape(C, TC, 128, PL).transpose(2, 1, 0, 3)
            ).reshape(128, TC, C * PL)
            yt = np.ascontiguousarray(yt_full[:, h * PL:(h + 1) * PL, :])
            in_maps.append({"xs": xk, "pi_d": pi_d, "offs": offs, "yt": yt})

    trace = bool(os.environ.get("BASS_TRACE"))
    LAST_RUN = run_bass_kernel_spmd(nc, in_maps, list(range(N_CORES)),
                                    trace=trace)
    C3 = np.concatenate(
        [LAST_RUN.results[2 * g]["c3"] + LAST_RUN.results[2 * g + 1]["c3"]
         for g in range(GX)], axis=0)

    # Host epilogue: rank-1 corrections (0.15% of FLOPs).
    row_c = pi.sum(-1)                                 # [C, T]
    col_c = pi.sum(1)                                  # [C, TP]
    SX = np.einsum("itd,itd->it", X, X)                # [NX, T]
    SY = np.einsum("jpd,jpd->jp", Y, Y)                # [NY, TP]
    C1 = np.einsum("it,it->i", SX, row_c[classe])      # [NX]
    C2 = col_c @ SY.T                                  # [C, NY]
    return (C1[:, None] + C2[classe] - 2.0 * C3).astype(np.float32)


# revision 6
# speedup vs baseline: 1.1868x; 1.1868x over previous
"""Fused OT-DTW l2 cost-matrix kernel for Trainium2 (8 NeuronCores, SPMD).

mat_cost[i,j] = sum_{t,p,d} pi[cl(i)][t,p] * (X[i,t,d] - Y[j,p,d])^2
             = C1[i] + C2[cl(i), j] - 2 * C3[i,j]

with C3[i,j] = sum_{p,d} XP[i,p,d] * Y[j,p,d],  XP[i] = X[i].T @ pi[cl(i)].
The device computes the heavy parts (XP: ~69 GFLOP, C3: ~137 GFLOP) in
fp8e4m3 (pi is 0/1 so fp8 is exact for it; X/Y quantization error washes
out over the 65536-term contraction). The tiny rank-1 corrections C1/C2
(<0.2% of FLOPs) are applied on the host in fp32.

Sharding (4 row-groups x 2 p-halves): core k = 2g + h takes 256 rows of X
and contraction half p in [256h, 256h+256). Each core emits the partial
C3 over its p-half; the host adds the two partials per group. This halves
the Y stream per core (33.5MB, now under the stage-B PE time) and splits
stage A across cores with zero duplication, putting per-core PE work at
the global fp8 roofline (~164us).

Both stages run fp8 DoubleRow (contraction 256/instr). The dual-fp8 ISA
forbids register-offset APs on the moving operand, so the per-row pi
class offsets are baked statically into the program: the host groups
rows into 256 quadruples of 4 same-class rows (one row per group per
slot, so all cores share one slot->class schedule; the <=24 per-class
remainder rows form <=6 mixed quadruples that fall back to values_load +
normal-mode matmuls). The program is compiled per classe (cache keyed on
the schedule); the host un-permutes output rows.

Stage A computes XPT_i[d,p_half] = X[i].T @ pi_cl(i) as 2 accumulating
DR matmuls per row (t-chunk pairs), then corner-turns PSUM->SBUF into
xpt[d, p, i] fp8 with packed 4-row casts split across DVE/ACT. Stage B
accumulates C3[i, j] over 128 DR p-pairs x 2 i-chunks x 2 j-halves:
lhsT = xpt[:, p:p+2, ic], rhs = yt tile [d, 8p, 1024j] streamed through
an 8-buffer ring. A short scratch-matmul burst first warms the PE
clock-gate.
"""

import os
import sys
import types

import numpy as np
import ml_dtypes

NX, NY, T, TP, D, C = 1024, 1024, 512, 512, 128, 8
N_CORES = 8
GX = 4                      # row groups
R = NX // GX                # 256 rows per core
PL = TP // 2                # 256 local p (contraction half)
TC = T // 128               # 4 t-chunks
PG = 8                      # p-tiles per Y DMA
BF16 = ml_dtypes.bfloat16
F8 = ml_dtypes.float8_e4m3fn


def _ensure_axon_hooks():
    """concourse.bass_utils imports antenv.axon_hooks when tracing under
    axon; some images lack that submodule. Provide it, and register the
    NTFF profile hook if the boot path didn't."""
    try:
        import antenv
    except ImportError:
        return
    try:
        from antenv import axon_hooks  # noqa: F401
    except ImportError:
        mod = types.ModuleType("antenv.axon_hooks")
        mod._hook = None

        def _set(h):
            mod._hook = h

        def _get():
            return mod._hook

        mod.set_axon_ntff_profile_hook = _set
        mod.get_axon_ntff_profile_hook = _get
        sys.modules["antenv.axon_hooks"] = mod
        antenv.axon_hooks = mod
    from antenv.axon_hooks import (
        get_axon_ntff_profile_hook,
        set_axon_ntff_profile_hook,
    )

    if get_axon_ntff_profile_hook() is None:
        try:
            from trn_agent_boot.trn_boot import _ntff_profile_via_ctypes

            hook = _ntff_profile_via_ctypes("/opt/axon/libaxon_pjrt.so")
            if hook is not None:
                set_axon_ntff_profile_hook(hook)
        except Exception:
            pass


_ensure_axon_hooks()

import concourse.bass as bass  # noqa: E402
import concourse.tile as tile  # noqa: E402
from concourse import bacc, mybir  # noqa: E402
from concourse.bass_utils import run_bass_kernel_spmd  # noqa: E402

_PROGRAM_CACHE = {}
LAST_RUN = None  # BassKernelResults of the most recent kernel() call


def _schedule(classe):
    """Group rows into 256 quadruples (one row per group per slot).

    Returns (slot_cls, perm): slot_cls[s] = class of slot s (or -1 for a
    mixed slot, handled via values_load), perm[s, g] = original row id
    placed at slot s of group g.
    """
    by_cls = [np.flatnonzero(classe == c) for c in range(C)]
    slot_cls, quads = [], []
    leftovers = []
    for c in range(C):
        rows = by_cls[c]
        n4 = len(rows) // 4 * 4
        for k in range(0, n4, 4):
            quads.append(rows[k:k + 4])
            slot_cls.append(c)
        leftovers.extend(rows[n4:])
    leftovers = np.asarray(leftovers, dtype=np.int64)
    assert len(leftovers) % 4 == 0
    for k in range(0, len(leftovers), 4):
        quads.append(leftovers[k:k + 4])
        slot_cls.append(-1)
    perm = np.stack(quads)                      # [256, 4]
    assert perm.shape == (R, GX)
    return tuple(slot_cls), perm


def _build_program(slot_cls):
    if slot_cls in _PROGRAM_CACHE:
        return _PROGRAM_CACHE[slot_cls]
    f8 = mybir.dt.float8e4
    f32 = mybir.dt.float32
    i32 = mybir.dt.int32
    DR = mybir.MatmulPerfMode.DoubleRow
    nc = bacc.Bacc("TRN2", target_bir_lowering=False, debug=False,
                   num_devices=N_CORES)
    xs = nc.dram_tensor("xs", [R, 128, TC, D], f8, kind="ExternalInput").ap()
    pi_d = nc.dram_tensor("pi_d", [128, TC, C * PL], f8, kind="ExternalInput").ap()
    offs = nc.dram_tensor("offs", [1, R], i32, kind="ExternalInput").ap()
    yt = nc.dram_tensor("yt", [D, PL, NY], f8, kind="ExternalInput").ap()
    c3 = nc.dram_tensor("c3", [R, NY], f32, kind="ExternalOutput").ap()

    with tile.TileContext(nc) as tc:
        with (
            tc.tile_pool(name="xpt", bufs=1) as xpt_pool,
            tc.tile_pool(name="xin", bufs=4) as xin_pool,
            tc.tile_pool(name="pisb", bufs=1) as pi_pool,
            tc.tile_pool(name="yin", bufs=8) as y_pool,
            tc.tile_pool(name="outsb", bufs=1) as out_pool,
        ):
            # Resident transposed XP for all local rows: [d, p, i] fp8
            # (p-major pairs for DoubleRow lhsT interleave).
            xpt = xpt_pool.tile([D, PL, R], f8)

            # PE warmup: ~14 matmuls on scratch data at t=0, overlapping the
            # first input DMAs, so the HAM clock-gate reaches 8/8 before the
            # real matmuls start (values never read; NaNs harmless).
            with (
                tc.tile_pool(name="warm", bufs=1) as warm_pool,
                tc.tile_pool(name="warmps", bufs=1, space="PSUM") as warmps_pool,
            ):
                wsrc = warm_pool.tile([128, 512], f8)
                wacc = warmps_pool.tile([128, 512], f32)
                nc.gpsimd.memset(wsrc[:], 0.0)
                for w in range(14):
                    nc.tensor.matmul(wacc[:], wsrc[:, 0:128], wsrc[:],
                                     start=True, stop=True)

            # ---- Stage A: XPT_i = X[i].T @ pi[cl(i)][:, p-half] ----
            # pi resident in SBUF (all classes' half, 1MB). Class per slot is
            # baked statically (dual-fp8 DR forbids register-offset moving
            # operands); mixed slots fall back to values_load + normal mode.
            off_sb = pi_pool.tile([1, R], i32)
            nc.sync.dma_start(off_sb[:], offs[:])
            pi_sb = pi_pool.tile([128, TC, C * PL], f8)
            for c in range(TC):   # per-chunk loads: first matmul only waits c=0
                nc.sync.dma_start(pi_sb[:, c, :], pi_d[:, c, :])
            XB, CB = 4, 4   # xs rows per DMA, rows per packed cast
            with tc.tile_pool(name="psA", bufs=2, space="PSUM") as psA_pool:
                for i in range(R):
                    if i % XB == 0:
                        xt = xin_pool.tile([128, XB, TC, D], f8, tag="xt")
                        nc.sync.dma_start(
                            xt[:], xs[i:i + XB].rearrange("a t c d -> t a c d"))
                    if i % CB == 0:
                        acc = psA_pool.tile([D, CB, PL], f32)  # 2 PSUM banks
                    xv = xt[:, i % XB]
                    if slot_cls[i] >= 0:
                        o = slot_cls[i] * PL
                        for h in range(TC // 2):
                            nc.tensor.matmul(
                                acc[:, i % CB, :],
                                xv[:, 2 * h:2 * h + 2, :],
                                pi_sb[:, 2 * h:2 * h + 2, o:o + PL],
                                start=(h == 0), stop=(h == TC // 2 - 1),
                                perf_mode=DR,
                            )
                    else:
                        off = nc.values_load(
                            off_sb[0:1, i:i + 1],
                            engines=[mybir.EngineType.PE],
                            min_val=0, max_val=(C - 1) * PL,
                            skip_runtime_bounds_check=True)
                        for cchunk in range(TC):
                            nc.tensor.matmul(
                                acc[:, i % CB, :],
                                xv[:, cchunk, :],
                                pi_sb[:, cchunk, bass.ds(off, PL)],
                                start=(cchunk == 0), stop=(cchunk == TC - 1),
                            )
                    if i % CB == CB - 1:
                        # Packed corner-turn: psum[d, 4, p] -> xpt[d, p, i..i+3]
                        # (4B inner runs @128B stride; 4x the 1B-run rate).
                        # Split p-range across DVE and ACT to halve latency.
                        g0 = i - (CB - 1)
                        h2 = PL // 2
                        s0 = acc[:, :, 0:h2].rearrange("d k p -> d p k")
                        s1 = acc[:, :, h2:PL].rearrange("d k p -> d p k")
                        nc.vector.tensor_copy(xpt[:, 0:h2, g0:g0 + CB], s0)
                        nc.scalar.copy(xpt[:, h2:PL, g0:g0 + CB], s1)

            # ---- Stage B: C3[i, j] partial = sum_{p half} XPT YT, DR pairs ----
            with tc.tile_pool(name="psB", bufs=1, space="PSUM") as psB_pool:
                accs = [[psB_pool.tile([128, 512], f32, name=f"accB_{ic}_{jh}")
                         for jh in range(2)]
                        for ic in range(2)]   # [i-chunk][j-half]
                for g in range(PL // PG):
                    ytile = y_pool.tile([D, PG, NY], f8)
                    nc.sync.dma_start(ytile[:], yt[:, g * PG:(g + 1) * PG, :])
                    for s in range(PG // 2):
                        p = g * PG + 2 * s
                        st, sp = (p == 0), (p == PL - 2)
                        rhs = ytile[:, 2 * s:2 * s + 2, :]
                        for ic in range(2):
                            lhsT = xpt[:, p:p + 2, 128 * ic:128 * ic + 128]
                            for jh in range(2):
                                nc.tensor.matmul(
                                    accs[ic][jh][:],
                                    lhsT, rhs[:, :, 512 * jh:512 * jh + 512],
                                    start=st, stop=sp, perf_mode=DR)

            out_sb = out_pool.tile([128, 2, NY], f32)
            nc.vector.tensor_copy(out_sb[:, 0, 0:512], accs[0][0][:])
            nc.scalar.copy(out_sb[:, 0, 512:1024], accs[0][1][:])
            nc.vector.tensor_copy(out_sb[:, 1, 0:512], accs[1][0][:])
            nc.scalar.copy(out_sb[:, 1, 512:1024], accs[1][1][:])
            nc.sync.dma_start(c3.rearrange("(ic q) j -> q ic j", q=128), out_sb[:])

    nc.compile()
    _PROGRAM_CACHE[slot_cls] = nc
    return nc


def kernel(X, Y, pi, classe):
    global LAST_RUN
    assert X.shape == (NX, T, D) and Y.shape == (NY, TP, D)
    assert pi.shape == (C, T, TP) and classe.shape == (NX,)
    X = np.asarray(X, dtype=np.float32)
    Y = np.asarray(Y, dtype=np.float32)
    pi = np.asarray(pi, dtype=np.float32)
    classe = np.asarray(classe)

    slot_cls, perm = _schedule(classe)
    nc = _build_program(slot_cls)

    # Host-side sharding + layout prep (all-contiguous device DMAs).
    yt_full = np.ascontiguousarray(Y.transpose(2, 1, 0).astype(F8))  # [d,p,j]
    pi8 = pi.astype(F8)
    in_maps = []
    for g in range(GX):
        rows = perm[:, g]
        xk = X[rows].astype(F8)                        # [R, T, D]
        xk = np.ascontiguousarray(
            xk.reshape(R, TC, 128, D).transpose(0, 2, 1, 3))
        offs = (classe[rows].astype(np.int32) * PL)[None, :]
        for h in range(2):
            # pi_d[t%128, c, cls*PL + p] = pi[cls, t, 256h + p]
            pi_d = np.ascontiguousarray(
                pi8[:, :, h * PL:(h + 1) * PL]
                .reshape(C, TC, 128, PL).transpose(2, 1, 0, 3)
            ).reshape(128, TC, C * PL)
            yt = np.ascontiguousarray(yt_full[:, h * PL:(h + 1) * PL, :])
            in_maps.append({"xs": xk, "pi_d": pi_d, "offs": offs, "yt": yt})

    trace = bool(os.environ.get("BASS_TRACE"))
    LAST_RUN = run_bass_kernel_spmd(nc, in_maps, list(range(N_CORES)),
                                    trace=trace)
    C3 = np.empty((NX, NY), np.float32)
    for g in range(GX):
        part = LAST_RUN.results[2 * g]["c3"] + LAST_RUN.results[2 * g + 1]["c3"]
        C3[perm[:, g]] = part

    # Host epilogue: rank-1 corrections (0.15% of FLOPs).
    row_c = pi.sum(-1)                                 # [C, T]
    col_c = pi.sum(1)                                  # [C, TP]
    SX = np.einsum("itd,itd->it", X, X)                # [NX, T]
    SY = np.einsum("jpd,jpd->jp", Y, Y)                # [NY, TP]
    C1 = np.einsum("it,it->i", SX, row_c[classe])      # [NX]
    C2 = col_c @ SY.T                                  # [C, NY]
    return (C1[:, None] + C2[classe] - 2.0 * C3).astype(np.float32)


# revision 8
# speedup vs baseline: 1.3862x; 1.1680x over previous
"""Fused OT-DTW l2 cost-matrix kernel for Trainium2 (8 NeuronCores, SPMD).

mat_cost[i,j] = sum_{t,p,d} pi[cl(i)][t,p] * (X[i,t,d] - Y[j,p,d])^2
             = C1[i] + C2[cl(i), j] - 2 * C3[i,j]

with C3[i,j] = sum_{p,d} XP[i,p,d] * Y[j,p,d],  XP[i] = X[i].T @ pi[cl(i)].
The device computes the heavy parts (XP: ~69 GFLOP, C3: ~137 GFLOP) in
fp8e4m3 (pi is 0/1 so fp8 is exact for it; X/Y quantization error washes
out over the 65536-term contraction). The tiny rank-1 corrections C1/C2
(<0.2% of FLOPs) are applied on the host in fp32.

Sharding (4 row-groups x 2 p-halves): core k = 2g + h takes 256 rows of X
and contraction half p in [256h, 256h+256). Each core emits the partial
C3 over its p-half; the host adds the two partials per group. This halves
the Y stream per core (33.5MB, now under the stage-B PE time) and splits
stage A across cores with zero duplication, putting per-core PE work at
the global fp8 roofline (~164us).

Both stages run fp8 DoubleRow (contraction 256/instr). The dual-fp8 ISA
forbids register-offset APs on the moving operand, so the per-row pi
class offsets are baked statically into the program: the host groups
rows into 256 quadruples of 4 same-class rows (one row per group per
slot, so all cores share one slot->class schedule; the <=24 per-class
remainder rows form <=6 mixed quadruples that fall back to values_load +
normal-mode matmuls). The program is compiled per classe (cache keyed on
the schedule); the host un-permutes output rows.

Stage A computes XPT_i[d,p_half] = X[i].T @ pi_cl(i) as 2 accumulating
DR matmuls per row (t-chunk pairs), then corner-turns PSUM->SBUF into
xpt[d, p, i] fp8 with packed 4-row casts split across DVE/ACT. Stage B
accumulates C3[i, j] over 128 DR p-pairs x 2 i-chunks x 2 j-halves:
lhsT = xpt[:, p:p+2, ic], rhs = yt tile [d, 8p, 1024j] streamed through
an 8-buffer ring. A short scratch-matmul burst first warms the PE
clock-gate.
"""

import os
import sys
import types

import numpy as np
import ml_dtypes

NX, NY, T, TP, D, C = 1024, 1024, 512, 512, 128, 8
N_CORES = 8
GX = 4                      # row groups
R = NX // GX                # 256 rows per core
PL = TP // 2                # 256 local p (contraction half)
TC = T // 128               # 4 t-chunks
PG = 8                      # p-tiles per Y DMA
BF16 = ml_dtypes.bfloat16
F8 = ml_dtypes.float8_e4m3fn


def _ensure_axon_hooks():
    """concourse.bass_utils imports antenv.axon_hooks when tracing under
    axon; some images lack that submodule. Provide it, and register the
    NTFF profile hook if the boot path didn't."""
    try:
        import antenv
    except ImportError:
        return
    try:
        from antenv import axon_hooks  # noqa: F401
    except ImportError:
        mod = types.ModuleType("antenv.axon_hooks")
        mod._hook = None

        def _set(h):
            mod._hook = h

        def _get():
            return mod._hook

        mod.set_axon_ntff_profile_hook = _set
        mod.get_axon_ntff_profile_hook = _get
        sys.modules["antenv.axon_hooks"] = mod
        antenv.axon_hooks = mod
    from antenv.axon_hooks import (
        get_axon_ntff_profile_hook,
        set_axon_ntff_profile_hook,
    )

    if get_axon_ntff_profile_hook() is None:
        try:
            from trn_agent_boot.trn_boot import _ntff_profile_via_ctypes

            hook = _ntff_profile_via_ctypes("/opt/axon/libaxon_pjrt.so")
            if hook is not None:
                set_axon_ntff_profile_hook(hook)
        except Exception:
            pass


_ensure_axon_hooks()

import concourse.bass as bass  # noqa: E402
import concourse.tile as tile  # noqa: E402
from concourse import bacc, mybir  # noqa: E402
from concourse.bass_utils import run_bass_kernel_spmd  # noqa: E402

_PROGRAM_CACHE = {}
LAST_RUN = None  # BassKernelResults of the most recent kernel() call


def _schedule(classe):
    """Group rows into 256 quadruples (one row per group per slot).

    Returns (slot_cls, perm): slot_cls[s] = class of slot s (or -1 for a
    mixed slot, handled via values_load), perm[s, g] = original row id
    placed at slot s of group g.
    """
    by_cls = [np.flatnonzero(classe == c) for c in range(C)]
    slot_cls, quads = [], []
    leftovers = []
    for c in range(C):
        rows = by_cls[c]
        n4 = len(rows) // 4 * 4
        for k in range(0, n4, 4):
            quads.append(rows[k:k + 4])
            slot_cls.append(c)
        leftovers.extend(rows[n4:])
    leftovers = np.asarray(leftovers, dtype=np.int64)
    assert len(leftovers) % 4 == 0
    for k in range(0, len(leftovers), 4):
        quads.append(leftovers[k:k + 4])
        slot_cls.append(-1)
    perm = np.stack(quads)                      # [256, 4]
    assert perm.shape == (R, GX)
    return tuple(slot_cls), perm


def _build_program(slot_cls):
    if slot_cls in _PROGRAM_CACHE:
        return _PROGRAM_CACHE[slot_cls]
    f8 = mybir.dt.float8e4
    f32 = mybir.dt.float32
    i32 = mybir.dt.int32
    DR = mybir.MatmulPerfMode.DoubleRow
    nc = bacc.Bacc("TRN2", target_bir_lowering=False, debug=False,
                   num_devices=N_CORES)
    xs = nc.dram_tensor("xs", [R, 128, TC, D], f8, kind="ExternalInput").ap()
    pi_d = nc.dram_tensor("pi_d", [128, TC, C * PL], f8, kind="ExternalInput").ap()
    offs = nc.dram_tensor("offs", [1, R], i32, kind="ExternalInput").ap()
    yt = nc.dram_tensor("yt", [D, PL, NY], f8, kind="ExternalInput").ap()
    c3 = nc.dram_tensor("c3", [R, NY], f32, kind="ExternalOutput").ap()

    with tile.TileContext(nc) as tc:
        with (
            tc.tile_pool(name="xpt", bufs=1) as xpt_pool,
            tc.tile_pool(name="xin", bufs=8) as xin_pool,
            tc.tile_pool(name="pisb", bufs=1) as pi_pool,
            tc.tile_pool(name="yin", bufs=6) as y_pool,
            tc.tile_pool(name="outsb", bufs=1) as out_pool,
        ):
            # Resident transposed XP for all local rows: [d, p, i] fp8
            # (p-major pairs for DoubleRow lhsT interleave).
            xpt = xpt_pool.tile([D, PL, R], f8)

            # PE warmup: ~14 matmuls on scratch data at t=0, overlapping the
            # first input DMAs, so the HAM clock-gate reaches 8/8 before the
            # real matmuls start (values never read; NaNs harmless).
            with (
                tc.tile_pool(name="warm", bufs=1) as warm_pool,
                tc.tile_pool(name="warmps", bufs=1, space="PSUM") as warmps_pool,
            ):
                wsrc = warm_pool.tile([128, 512], f8)
                wacc = warmps_pool.tile([128, 512], f32)
                nc.gpsimd.memset(wsrc[:], 0.0)
                for w in range(14):
                    nc.tensor.matmul(wacc[:], wsrc[:, 0:128], wsrc[:],
                                     start=True, stop=True)

            # ---- Stage A: XPT_i = X[i].T @ pi[cl(i)][:, p-half] ----
            # pi resident in SBUF (all classes' half, 1MB). Class per slot is
            # baked statically (dual-fp8 DR forbids register-offset moving
            # operands); mixed slots fall back to values_load + normal mode.
            off_sb = pi_pool.tile([1, R], i32)
            nc.sync.dma_start(off_sb[:], offs[:])
            pi_sb = pi_pool.tile([128, TC, C * PL], f8)
            for c in range(TC):   # per-chunk loads: first matmul only waits c=0
                nc.sync.dma_start(pi_sb[:, c, :], pi_d[:, c, :])
            XB, CB = 8, 4   # xs rows per DMA, rows per packed cast
            with tc.tile_pool(name="psA", bufs=2, space="PSUM") as psA_pool:
                for i in range(R):
                    if i % XB == 0:
                        xt = xin_pool.tile([128, XB, TC, D], f8, tag="xt")
                        nc.sync.dma_start(
                            xt[:], xs[i:i + XB].rearrange("a t c d -> t a c d"))
                    if i % CB == 0:
                        acc = psA_pool.tile([D, CB, PL], f32)  # 2 PSUM banks
                    xv = xt[:, i % XB]
                    if slot_cls[i] >= 0:
                        o = slot_cls[i] * PL
                        for h in range(TC // 2):
                            nc.tensor.matmul(
                                acc[:, i % CB, :],
                                xv[:, 2 * h:2 * h + 2, :],
                                pi_sb[:, 2 * h:2 * h + 2, o:o + PL],
                                start=(h == 0), stop=(h == TC // 2 - 1),
                                perf_mode=DR,
                            )
                    else:
                        off = nc.values_load(
                            off_sb[0:1, i:i + 1],
                            engines=[mybir.EngineType.PE],
                            min_val=0, max_val=(C - 1) * PL,
                            skip_runtime_bounds_check=True)
                        for cchunk in range(TC):
                            nc.tensor.matmul(
                                acc[:, i % CB, :],
                                xv[:, cchunk, :],
                                pi_sb[:, cchunk, bass.ds(off, PL)],
                                start=(cchunk == 0), stop=(cchunk == TC - 1),
                            )
                    if i % CB == CB - 1:
                        # Packed corner-turn: psum[d, 4, p] -> xpt[d, p, i..i+3]
                        # (4B inner runs @128B stride; 4x the 1B-run rate).
                        # Split p-range across DVE and ACT to halve latency.
                        g0 = i - (CB - 1)
                        h2 = PL // 2
                        s0 = acc[:, :, 0:h2].rearrange("d k p -> d p k")
                        s1 = acc[:, :, h2:PL].rearrange("d k p -> d p k")
                        nc.vector.tensor_copy(xpt[:, 0:h2, g0:g0 + CB], s0)
                        nc.scalar.copy(xpt[:, h2:PL, g0:g0 + CB], s1)

            # ---- Stage B: C3[i, j] partial = sum_{p half} XPT YT, DR pairs ----
            with tc.tile_pool(name="psB", bufs=1, space="PSUM") as psB_pool:
                accs = [[psB_pool.tile([128, 512], f32, name=f"accB_{ic}_{jh}")
                         for jh in range(2)]
                        for ic in range(2)]   # [i-chunk][j-half]
                for g in range(PL // PG):
                    ytile = y_pool.tile([D, PG, NY], f8)
                    nc.sync.dma_start(ytile[:], yt[:, g * PG:(g + 1) * PG, :])
                    for s in range(PG // 2):
                        p = g * PG + 2 * s
                        st, sp = (p == 0), (p == PL - 2)
                        rhs = ytile[:, 2 * s:2 * s + 2, :]
                        for ic in range(2):
                            lhsT = xpt[:, p:p + 2, 128 * ic:128 * ic + 128]
                            for jh in range(2):
                                nc.tensor.matmul(
                                    accs[ic][jh][:],
                                    lhsT, rhs[:, :, 512 * jh:512 * jh + 512],
                                    start=st, stop=sp, perf_mode=DR)

            out_sb = out_pool.tile([128, 2, NY], f32)
            nc.vector.tensor_copy(out_sb[:, 0, 0:512], accs[0][0][:])
            nc.scalar.copy(out_sb[:, 0, 512:1024], accs[0][1][:])
            nc.vector.tensor_copy(out_sb[:, 1, 0:512], accs[1][0][:])
            nc.scalar.copy(out_sb[:, 1, 512:1024], accs[1][1][:])
            nc.sync.dma_start(c3.rearrange("(ic q) j -> q ic j", q=128), out_sb[:])

    nc.compile()
    _PROGRAM_CACHE[slot_cls] = nc
    return nc


def kernel(X, Y, pi, classe):
    global LAST_RUN
    assert X.shape == (NX, T, D) and Y.shape == (NY, TP, D)
    assert pi.shape == (C, T, TP) and classe.shape == (NX,)
    X = np.asarray(X, dtype=np.float32)
    Y = np.asarray(Y, dtype=np.float32)
    pi = np.asarray(pi, dtype=np.float32)
    classe = np.asarray(classe)

    slot_cls, perm = _schedule(classe)
    nc = _build_program(slot_cls)

    # Host-side sharding + layout prep (all-contiguous device DMAs).
    yt_full = np.ascontiguousarray(Y.transpose(2, 1, 0).astype(F8))  # [d,p,j]
    pi8 = pi.astype(F8)
    in_maps = []
    for g in range(GX):
        rows = perm[:, g]
        xk = X[rows].astype(F8)                        # [R, T, D]
        xk = np.ascontiguousarray(
            xk.reshape(R, TC, 128, D).transpose(0, 2, 1, 3))
        offs = (classe[rows].astype(np.int32) * PL)[None, :]
        for h in range(2):
            # pi_d[t%128, c, cls*PL + p] = pi[cls, t, 256h + p]
            pi_d = np.ascontiguousarray(
                pi8[:, :, h * PL:(h + 1) * PL]
                .reshape(C, TC, 128, PL).transpose(2, 1, 0, 3)
            ).reshape(128, TC, C * PL)
            yt = np.ascontiguousarray(yt_full[:, h * PL:(h + 1) * PL, :])
            in_maps.append({"xs": xk, "pi_d": pi_d, "offs": offs, "yt": yt})

    trace = bool(os.environ.get("BASS_TRACE"))
    LAST_RUN = run_bass_kernel_spmd(nc, in_maps, list(range(N_CORES)),
                                    trace=trace)
    C3 = np.empty((NX, NY), np.float32)
    for g in range(GX):
        part = LAST_RUN.results[2 * g]["c3"] + LAST_RUN.results[2 * g + 1]["c3"]
        C3[perm[:, g]] = part

    # Host epilogue: rank-1 corrections (0.15% of FLOPs).
    row_c = pi.sum(-1)                                 # [C, T]
    col_c = pi.sum(1)                                  # [C, TP]
    SX = np.einsum("itd,itd->it", X, X)                # [NX, T]
    SY = np.einsum("jpd,jpd->jp", Y, Y)                # [NY, TP]
    C1 = np.einsum("it,it->i", SX, row_c[classe])      # [NX]
    C2 = col_c @ SY.T                                  # [C, NY]
    return (C1[:, None] + C2[classe] - 2.0 * C3).astype(np.float32)
